# revision 9
# baseline (speedup 1.0000x reference)
"""Grouped MLP (64 independent 512x1024 @ 1024x1024 GEMMs + bias) on 8 trn2 cores.

out[b, r, o] = sum_i x[b, r, i] * W[r, i, o] + bias[r, o]
  x: (512, 64, 1024) f32, W: (64, 1024, 1024) f32, bias: (64, 1024) f32

Sharding: expert-parallel over the row dim (64 rows -> 8 per core).

Per-core compute: 8 GEMMs of [512,1024]x[1024,1024] in bf16 = 8.59 GFLOP
-> 109 us at the 78.6 TF/s PE roofline; HBM traffic 33.6 MB -> 94 us at
358 GB/s. Compute-bound; the kernel streams one fused [xT | W] 384 KB
block per (row, k-tile) through a single HWDGE queue (FIFO, full rate)
and keeps the PE issuing N=512 matmuls back to back.

Layout: out_dim on PSUM partitions (stationary = W k-slice [128k, 128o],
moving = xT [128k, 512b]), so bias is a per-partition scalar in the
epilogue: ACT/DVE split the PSUM->SBUF bias-add by bank, scalar-engine
HWDGE stores each [128, 512] bank. Rows 0-6 run k-major (so row 0 can
start after one 384 KB block); row 7 runs bank-major so only one bank's
drain+store is exposed after the last matmul. Dummy matmuls on zeros
warm the PE clock (HAM) during the initial DMA fill.

Host-side prep (off the device clock): pack x^T and W into the fused
[row, k, 128, 512+1024] bf16 blocks, bias into [128, row*otile]; output
returns as [row, otile, 128, 512] bf16 and is unscrambled + upcast.
"""

import numpy as np

ROW, IN_DIM, OUT_DIM, BATCH = 64, 1024, 1024, 512
N_CORES = 8
R_PER_CORE = ROW // N_CORES  # 8
P = 128
K_TILES = IN_DIM // P  # 8
O_TILES = OUT_DIM // P  # 8
XW_COLS = BATCH + OUT_DIM  # 1536
XW_BUFS = 20  # rotating SBUF blocks: row-resident (8) + 1-row-ahead prefetch
N_WARMUP = 30  # dummy N=128 matmuls to lift the PE clock gate during DMA fill

_cached = {}


def _build_program(loop_T=None):
    import concourse.bacc as bacc
    import concourse.mybir as mybir
    import concourse.tile as tile
    import contextlib

    bf16 = mybir.dt.bfloat16

    nc = bacc.Bacc(
        "TRN2", target_bir_lowering=False, debug=False, num_devices=N_CORES
    )
    XW = nc.declare_dram_parameter(
        "XW", [R_PER_CORE, K_TILES, P, XW_COLS], bf16, isOutput=False
    )
    BIASP = nc.declare_dram_parameter(
        "biasP", [P, R_PER_CORE * O_TILES], mybir.dt.float32, isOutput=False
    )
    OUT = nc.declare_dram_parameter(
        "out", [R_PER_CORE, O_TILES, P, BATCH], bf16, isOutput=True
    )

    with tile.TileContext(nc) as tc:
        with (
            tc.tile_pool(name="xwpool", bufs=XW_BUFS) as xwpool,
            tc.tile_pool(name="opool", bufs=32) as opool,
            tc.tile_pool(name="cpool", bufs=1) as cpool,
            tc.tile_pool(name="psum", bufs=1, space="PSUM") as psum,
        ):
            loop_cm = (
                tc.For_i(0, loop_T, 1)
                if loop_T is not None
                else contextlib.nullcontext()
            )
            with loop_cm:
                # PE warm-up on zeros while the first input blocks stream
                # in: short N=128 matmuls keep the clock-gate activity
                # window busy without delaying the first real matmul.
                wu = cpool.tile([P, BATCH], bf16, name="wu")
                nc.vector.memset(wu[:], 0.0)
                wu_ps = psum.tile(
                    [P, BATCH], mybir.dt.float32, tag="ps7", name="wu_ps"
                )
                for i in range(N_WARMUP):
                    nc.tensor.matmul(
                        wu_ps[:, :P], wu[:, :P], wu[:, :P],
                        start=True, stop=True,
                    )

                bias_sb = cpool.tile(
                    [P, R_PER_CORE * O_TILES], mybir.dt.float32, name="bias_sb"
                )

                def xw_dma(r, k):
                    t = xwpool.tile(
                        [P, XW_COLS], bf16, tag="xw", name=f"xw_{r}_{k}"
                    )
                    if r == 0 and k == 0:
                        # first block rides the (otherwise idle) SWDGE
                        # queue so it overlaps the sync queue's ramp-up
                        nc.gpsimd.dma_start(t[:], XW[r, k])
                    else:
                        nc.sync.dma_start(t[:], XW[r, k])
                    return t

                def mm(ps_t, t, ot, k):
                    nc.tensor.matmul(
                        ps_t[:],
                        t[:, BATCH + ot * P : BATCH + (ot + 1) * P],
                        t[:, :BATCH],
                        start=(k == 0),
                        stop=(k == K_TILES - 1),
                    )

                pending_outs = []

                def epilogue(r, ot, ps_t, defer=True):
                    o_sb = opool.tile(
                        [P, BATCH], bf16, tag="o", name=f"o_{r}_{ot}"
                    )
                    bias_col = bias_sb[:, r * O_TILES + ot : r * O_TILES + ot + 1]
                    if ot % 2 == 0:
                        nc.vector.tensor_scalar_add(o_sb[:], ps_t[:], bias_col)
                    else:
                        nc.scalar.add(o_sb[:], ps_t[:], bias_col)
                    if defer:
                        # hold the store dispatch so the write queue does
                        # not steal SDMA bandwidth from the input stream
                        # while it is still the critical path
                        pending_outs.append((r, ot, o_sb))
                    else:
                        nc.scalar.dma_start(OUT[r, ot], o_sb[:])

                def flush_out(n=1):
                    for _ in range(min(n, len(pending_outs))):
                        r, ot, o_sb = pending_outs.pop(0)
                        nc.scalar.dma_start(OUT[r, ot], o_sb[:])

                # Fill-phase rows (0-2) are hybrid: half A runs k-major
                # (consumes block k as it lands, no up-front wait), half B
                # runs otile-major on the then-resident blocks. Steady-state
                # rows (3+) are fully otile-major: one bank accumulates its
                # 8 MMs back-to-back, banks complete staggered 1.7 us apart
                # so each has ~12 us of drain slack before next-row reuse.
                N_HYBRID = 3

                def emit_row_hybrid(r):
                    tiles = []
                    ps_h = [
                        psum.tile(
                            [P, BATCH], mybir.dt.float32,
                            tag=f"ps{ot}", name=f"ps_{r}_{ot}",
                        )
                        for ot in range(4)
                    ]
                    for k in range(K_TILES):
                        t = xw_dma(r, k)
                        tiles.append(t)
                        if r == 0 and k == 3:
                            # bias is first needed by row 0's epilogue at
                            # ~17us; keep it behind the first blocks.
                            nc.sync.dma_start(bias_sb[:], BIASP[:, :])
                        for ot in range(4):
                            mm(ps_h[ot], t, ot, k)
                    for ot in range(4):
                        epilogue(r, ot, ps_h[ot])
                    for ot in range(4, O_TILES):
                        ps_t = psum.tile(
                            [P, BATCH], mybir.dt.float32,
                            tag=f"ps{ot}", name=f"ps_{r}_{ot}",
                        )
                        for k in range(K_TILES):
                            mm(ps_t, tiles[k], ot, k)
                        epilogue(r, ot, ps_t)

                def emit_row_otmajor(r):
                    tiles = [xw_dma(r, k) for k in range(K_TILES)]
                    prompt = r >= R_PER_CORE - 2  # protect the tail
                    last = r == R_PER_CORE - 1
                    for ot in range(O_TILES):
                        ps_t = psum.tile(
                            [P, BATCH], mybir.dt.float32,
                            tag=f"ps{ot}", name=f"ps_{r}_{ot}",
                        )
                        for k in range(K_TILES):
                            mm(ps_t, tiles[k], ot, k)
                        if last and ot == O_TILES - 1:
                            # final bank: split the store into two DMAs on
                            # separate HWDGE queues to overlap the HBM
                            # write receipt at the very end of the kernel
                            o_sb = opool.tile(
                                [P, BATCH], bf16, tag="o", name=f"o_{r}_{ot}"
                            )
                            bc = bias_sb[:, r * O_TILES + ot : r * O_TILES + ot + 1]
                            nc.scalar.add(o_sb[:], ps_t[:], bc)
                            nc.scalar.dma_start(OUT[r, ot, :, : BATCH // 2],
                                                o_sb[:, : BATCH // 2])
                            nc.sync.dma_start(OUT[r, ot, :, BATCH // 2 :],
                                              o_sb[:, BATCH // 2 :])
                        else:
                            epilogue(r, ot, ps_t, defer=not prompt)
                        flush_out(2)
                    if prompt:
                        flush_out(8)

                for r in range(R_PER_CORE):
                    if r < N_HYBRID:
                        emit_row_hybrid(r)
                    else:
                        emit_row_otmajor(r)
                flush_out(len(pending_outs))

    nc.compile()
    return nc


def _np_bf16():
    import ml_dtypes

    return ml_dtypes.bfloat16


def _in_maps(x, W, b):
    bf16 = _np_bf16()
    maps = []
    for c in range(N_CORES):
        rs = slice(c * R_PER_CORE, (c + 1) * R_PER_CORE)
        # XW[r, k, p, 0:512] = x[b, r, k*128+p]; XW[r, k, p, 512:1536] = W[r, k*128+p, o]
        xr = np.ascontiguousarray(
            np.transpose(np.asarray(x[:, rs, :], dtype=np.float32), (1, 2, 0))
        ).reshape(R_PER_CORE, K_TILES, P, BATCH)
        wr = np.asarray(W[rs], dtype=np.float32).reshape(
            R_PER_CORE, K_TILES, P, OUT_DIM
        )
        xw = np.concatenate([xr, wr], axis=3).astype(bf16)
        # biasP[p, r*8+ot] = b[r, ot*128+p]
        bp = np.ascontiguousarray(
            np.asarray(b[rs], dtype=np.float32)
            .reshape(R_PER_CORE, O_TILES, P)
            .transpose(2, 0, 1)
            .reshape(P, R_PER_CORE * O_TILES)
        ).astype(np.float32)
        maps.append({"XW": xw, "biasP": bp})
    return maps


def _unscramble(out_cores):
    # per core: [R, O_TILES, P, BATCH] -> [BATCH, R, OUT_DIM]; concat rows
    full = []
    for oc in out_cores:
        o = np.asarray(oc).astype(np.float32)
        full.append(
            np.transpose(o, (3, 0, 1, 2)).reshape(BATCH, R_PER_CORE, OUT_DIM)
        )
    return np.concatenate(full, axis=1)


def _run(x, W, b, trace=False, variant=None, **trace_kwargs):
    from concourse.bass_utils import run_bass_kernel_spmd

    key = "main"
    if key not in _cached:
        _cached[key] = _build_program()
    nc = _cached[key]
    return run_bass_kernel_spmd(
        nc, _in_maps(x, W, b), list(range(N_CORES)),
        trace=trace, **trace_kwargs
    )


def kernel(x: np.ndarray, W: np.ndarray, b: np.ndarray) -> np.ndarray:
    res = _run(x, W, b)
    return _unscramble([res.results[c]["out"] for c in range(N_CORES)])


def run_profiled(x, W, b, variant=None):
    res = _run(x, W, b, trace=True, variant=variant)
    return {
        "exec_time_ns": res.exec_time_ns,
        "mean_exec_time_ns": res.mean_exec_time_ns,
        "profile_json": res.profile_json,
        "results": res,
    }
